# revision 63
# baseline (speedup 1.0000x reference)
"""Trainium2 Bass kernel for DiagonalKernelAverageV2.

Math: for each (b, ch) image X [512, 512] and each of 4 corners, the output
at index i is the mean over the L-shaped shell of the i-th nested corner
square:  shell[i] = d[i] - d[i-1],  d[i] = sum of the (i+1)x(i+1) corner
window,  counts[i] = 2i+1.

Only two shell families are computed directly (top-left and top-right); the
bottom corners follow from row/col totals:
    shell_tl[k] = sum_{c<=k} X[k,c] + sum_{r<k}  X[r,k]
    shell_tr[k] = sum_{c>=511-k} X[k,c] + sum_{r<k} X[r,511-k]
    shell_br[k] = S[k] + ST[k] - shell_tl[k]   (written in source order)
    shell_bl[k] = S[k] + STrev[k] - shell_tr[k]
(S = row totals, ST/STrev = col totals at col k / 511-k.)

Per-core layout: batch-sharded (4 batches x 8 channels per core).  Each image
is 4 row-tiles [128, 512] (partition p = row within tile t, k = 128t+p).

Engine split (all free-axis reductions must live on DVE/Act -- GPSIMD has no
reduce, no PSUM port, and no TensorScalarPtr on real TRN2):
  - DVE (bottleneck, ~2.6us/image): block row sums B[t][j] for 11 of the 16
    128x128 blocks via rectangular strided tensor_reduces, plus a strided
    reduce over the masked products PP -> strict row sums RS.  Reduces are
    merged across image groups (1/2/4 images per instruction, coarser as the
    DVE backlog builds) to amortize per-instruction init without starving on
    the group's last DMA arrival.
  - Activation: the other 5 block sums (diag x4, antidiag t=3) via the
    activation accumulator; PSUM->SBUF staging copies (batch-granular to
    amortize the PSUM access latency); half the input DMAs' companion queue.
  - Pool (GPSIMD): masked products PP[t,0] = diag block * msu (strict upper)
    and PP[t,1] = antidiag block * manti (strict anti-triangle); the batched
    assembly/weighting ops; half of the input DMAs.
  - SP: the other half of the input DMAs, consts, output DMAs (deferred by
    one batch so the input stream never stalls on a compute tail).
  - PE: per (g, t) PSUM accumulation groups of tiny fp32 matmuls with the
    128x128 X blocks / masked products as STATIONARY and 2-wide 0/1 weight
    columns moving:
      Qd[:,t,{0,1}] = {col-prefix above diag + strict in-block col prefix, ct}
      QA[:,t,{0,1}] = {anti col-prefix + strict anti col prefix, ct-rev}
        (both accumulated in antidiag-local coordinates, i.e. indexed by
         c_loc = 127-p), then
      QR = J @ QA  with J the 128x128 exchange matrix -- a free partition
        flip on the PE, since negative-stride matmul weights are illegal.
    The quantities land already transposed (partition = k), so no PE
    transposes and no wide fp32 matmuls are needed.
Bottom-corner outputs are written in source order and flipped on the host.
"""

import numpy as np

SIZE = 512
NT = 4  # row tiles per image
NCH = 8  # channels per batch
NB_CORE = 4  # batches per core
N_CORES = 8
NQ = 4  # quantity columns: 0=tl colpart, 1=tr colpart, 2=ct, 3=ctrev


def build_nc():
    import concourse.bass as bass
    import concourse.bacc as bacc
    import concourse.mybir as mybir
    from concourse.tile import TileContext

    f32 = mybir.dt.float32
    nc = bacc.Bacc()

    x = nc.dram_tensor("x", [NB_CORE, NCH, SIZE, SIZE], f32, kind="ExternalInput")
    # constants packed in one tensor: msu | manti | J | wq | e10 | wg | wrevg
    NC_CONST = 3 * 128 + 2 * NT * NT * 2 + 2 + 2 * NCH * NT
    cst_d = nc.dram_tensor("cst", [128, NC_CONST], f32, kind="ExternalInput")
    out = nc.dram_tensor("out", [NB_CORE, SIZE, 4 * NCH], f32, kind="ExternalOutput")

    ADD = mybir.AluOpType.add
    MULT = mybir.AluOpType.mult
    SUB = mybir.AluOpType.subtract
    AX = mybir.AxisListType.X

    with TileContext(nc) as tc:
        with (
            tc.tile_pool(name="consts", bufs=1) as consts,
            tc.tile_pool(name="xs", bufs=3) as xpool,
            tc.tile_pool(name="pp", bufs=3) as ppool,
            tc.tile_pool(name="perb", bufs=2) as bpool,
            tc.tile_pool(name="small", bufs=2) as spool,
            tc.tile_pool(name="psq", bufs=2, space="PSUM") as psq,
        ):
            cst = consts.tile([128, NC_CONST], f32)
            nc.sync.dma_start(out=cst, in_=cst_d[:])
            o_msu = 0
            o_manti = o_msu + 128
            o_J = o_manti + 128
            o_wq = o_J + 128
            o_e10 = o_wq + 2 * NT * NT * 2
            o_wg = o_e10 + 2
            o_wrevg = o_wg + NCH * NT
            msu = cst[:, o_msu : o_msu + 128]
            manti = cst[:, o_manti : o_manti + 128]
            Jx = cst[:, o_J : o_J + 128]
            wq = cst[:, o_wq : o_wq + 2 * NT * NT * 2].rearrange(
                "p (a t u q) -> p a t u q", a=2, t=NT, u=NT
            )
            e10 = cst[:, o_e10 : o_e10 + 2]
            wg = cst[:, o_wg : o_wg + NCH * NT].rearrange("p (g t) -> p g t", g=NCH)
            wrevg = cst[:, o_wrevg : o_wrevg + NCH * NT].rearrange(
                "p (g t) -> p g t", g=NCH
            )

            # output DMAs are deferred by one batch so the SP sequencer never
            # blocks the input-DMA stream waiting on a batch's compute tail
            pending_out = []

            for b in range(NB_CORE):
                B_G = bpool.tile([128, NCH, NT, NT], f32, tag="bg")
                RS = bpool.tile([128, NCH, NT, 2], f32, tag="rs")
                QB = bpool.tile([128, NCH, NT, NQ], f32, tag="qb")
                # per-batch PSUM accumulators (one bank each); staged to SBUF
                # with a few large Activation copies per batch (GPSIMD has no
                # PSUM port, and per-image Act copies pay the PSUM-access
                # latency 8x)
                QDR = psq.tile([128, NCH, NT, 4], f32, tag="qdr")
                QA = psq.tile([128, NCH, NT, 2], f32, tag="qa")
                Qd = QDR[:, :, :, 0:2]
                QR = QDR[:, :, :, 2:4]

                # reduce groups: interior quads are reduced in single
                # merged DVE instructions (amortizes per-instruction init on
                # the bottleneck engine); the first/last images stay
                # fine-grained to keep the pipeline lead-in short and the
                # drain chain incremental
                if b == 0:
                    group_end = {
                        0: (0, 1), 1: (1, 1), 3: (2, 2), 5: (4, 2), 7: (6, 2)
                    }
                elif b == 1:
                    group_end = {3: (0, 4), 7: (4, 4)}
                elif b == NB_CORE - 1:
                    group_end = {3: (0, 4), 7: (4, 4)}
                else:
                    group_end = {3: (0, 4), 7: (4, 4)}

                def b_reduces(X4, g0, i0, n):
                    # antidiag t=0..2 (affine lattice, manual AP)
                    nc.vector.tensor_reduce(
                        out=bass.AP(
                            tensor=B_G.tensor,
                            offset=B_G[:, g0, 0, 3:4].offset,
                            ap=[B_G[:, g0, 0, 3:4].ap[0]]
                            + [[16, n], [3, 3], [1, 1]],
                        ),
                        in_=bass.AP(
                            tensor=X4.tensor,
                            offset=X4[:, i0, 0, 384:385].offset,
                            ap=[X4[:, i0, 0, 384:385].ap[0]]
                            + [[NT * SIZE, n], [384, 3], [1, 128]],
                        ),
                        axis=AX,
                        op=ADD,
                    )
                    nc.vector.tensor_reduce(
                        out=B_G[:, g0 : g0 + n, 1:3, 0::3],
                        in_=X4[:, i0 : i0 + n, 1:3, :].rearrange(
                            "p i t (j c) -> p i t j c", c=128
                        )[:, :, :, 0::3, :],
                        axis=AX,
                        op=ADD,
                    )
                    nc.vector.tensor_reduce(
                        out=B_G[:, g0 : g0 + n, 0::3, 1:3],
                        in_=X4[:, i0 : i0 + n, 0::3, 128:384].rearrange(
                            "p i t (j c) -> p i t j c", c=128
                        ),
                        axis=AX,
                        op=ADD,
                    )

                if b == 0:
                    # pre-issue image 1's DMA on the (empty) Activation queue
                    # before any digsum is emitted there: image 1 then lands
                    # ~1us earlier than via the two shared input queues
                    X4_pre = xpool.tile([128, 4, NT, SIZE], f32, tag="x4")
                    nc.scalar.dma_start(
                        out=X4_pre[:, 1],
                        in_=x[b, 1].rearrange("(t p) c -> p t c", p=128),
                    )

                for g in range(NCH):
                    if g % 4 == 0:
                        # four images share one tile so the DVE reduces can
                        # cover a whole quad per instruction
                        if b == 0 and g == 0:
                            X4 = X4_pre
                        else:
                            X4 = xpool.tile([128, 4, NT, SIZE], f32, tag="x4")
                        PP4 = ppool.tile([128, 4, NT, 2, 128], f32, tag="pp4")
                    X = X4[:, g % 4]
                    # alternate the issuing queue: each DMA's transfer time
                    # occupies the issuing engine in the cost model, so
                    # splitting across SP and Pool doubles DMA throughput
                    idx = b * NCH + g
                    dma_eng = nc.gpsimd if idx % 3 == 0 else nc.sync
                    if idx == 1:
                        pass  # pre-issued on the Activation queue above
                    elif idx in (0, NB_CORE * NCH - 2, NB_CORE * NCH - 1):
                        # split first/last images per tile: compute starts
                        # earlier at the head and overlaps the DMA at the
                        # tail; alternating queues also fine-balances SP/Pool
                        xv = x[b, g].rearrange("(t p) c -> p t c", p=128)
                        for t4 in range(NT):
                            eng4 = nc.sync if (idx + t4) % 2 == 0 else nc.gpsimd
                            eng4.dma_start(out=X[:, t4], in_=xv[:, t4])
                    else:
                        dma_eng.dma_start(
                            out=X, in_=x[b, g].rearrange("(t p) c -> p t c", p=128)
                        )
                    if g in group_end:
                        g0, n = group_end[g]
                        b_reduces(X4, g0, g0 % 4, n)
                    # masked products on Pool (plain tensor_tensor mult with
                    # triangle masks; GPSIMD supports no fused accumulation):
                    # PP[t,0] = diag block * msu (strict upper),
                    # PP[t,1] = antidiag block * manti (strict anti-triangle).
                    # Their row sums come from strided DVE reduces per group.
                    PP = PP4[:, g % 4]
                    scr_act = ppool.tile([128, 128], f32, tag="scr")
                    for t in range(NT):
                        nc.gpsimd.tensor_tensor(
                            PP[:, t, 0],
                            X[:, t, 128 * t : 128 * (t + 1)],
                            msu,
                            op=MULT,
                        )
                        nc.gpsimd.tensor_tensor(
                            PP[:, t, 1],
                            X[:, t, 128 * (3 - t) : 128 * (4 - t)],
                            manti,
                            op=MULT,
                        )
                        # diag digsums (and antidiag of t=3) on Activation via
                        # the accumulator
                        nc.scalar.activation(
                            out=scr_act,
                            in_=X[:, t, 128 * t : 128 * (t + 1)],
                            func=mybir.ActivationFunctionType.Copy,
                            accum_out=B_G[:, g, t, t : t + 1],
                        )
                        if t == 3:
                            nc.scalar.activation(
                                out=scr_act,
                                in_=X[:, 3, 0:128],
                                func=mybir.ActivationFunctionType.Copy,
                                accum_out=B_G[:, g, 3, 0:1],
                            )
                    # masked row sums: strided DVE reduce over PP per group
                    # (split in two for the final image to shorten the drain)
                    if g in group_end:
                        g0, n = group_end[g]
                        i0 = g0 % 4
                        if idx == NB_CORE * NCH - 1 and n == 1:
                            nc.vector.tensor_reduce(
                                out=RS[:, g, 0:2], in_=PP[:, 0:2], axis=AX, op=ADD
                            )
                            nc.vector.tensor_reduce(
                                out=RS[:, g, 2:4], in_=PP[:, 2:4], axis=AX, op=ADD
                            )
                        else:
                            nc.vector.tensor_reduce(
                                out=RS[:, g0 : g0 + n],
                                in_=PP4[:, i0 : i0 + n],
                                axis=AX,
                                op=ADD,
                            )
                    # column-side quantities, all with positive-stride APs
                    # (the BIR verifier bans negative-stride matmul weights):
                    #   Qd[:,g,t,{0,1}] = {tl col part, ct}         (direct)
                    #   QA[:,g,t,{0,1}] = {anti prefix, ctrev} indexed by c_loc
                    #   QR = colsum(PP2) now + J @ QA (partition flip) at
                    #        batch end -> {tr, ctrev}
                    for t in range(NT):
                        for tp in range(NT):
                            nc.tensor.matmul(
                                Qd[:, g, t, :],
                                lhsT=X[:, tp, 128 * t : 128 * (t + 1)],
                                rhs=wq[:, 0, t, tp],
                                start=(tp == 0),
                                stop=False,
                            )
                        nc.tensor.matmul(
                            Qd[:, g, t, :], lhsT=PP[:, t, 0], rhs=e10,
                            start=False, stop=True,
                        )
                    for t in range(NT):
                        for tp in range(NT):
                            nc.tensor.matmul(
                                QA[:, g, t, :],
                                lhsT=X[:, tp, 128 * (3 - t) : 128 * (4 - t)],
                                rhs=wq[:, 1, t, tp],
                                start=(tp == 0),
                                stop=False,
                            )
                        nc.tensor.matmul(
                            QA[:, g, t, :], lhsT=PP[:, t, 1], rhs=e10,
                            start=False, stop=True,
                        )

                while pending_out:
                    dst, osrc, oeng = pending_out.pop(0)
                    oeng.dma_start(out=dst, in_=osrc)

                # batch-end: stage QA, flip partitions via J matmuls into QR
                QAs = spool.tile([128, NCH, NT, 2], f32, tag="qas")
                if b < NB_CORE - 1:
                    nc.scalar.copy(
                        QAs.rearrange("p g t q -> p (g t q)"),
                        QA.rearrange("p g t q -> p (g t q)"),
                    )
                else:
                    # final batch: stage per pair of images as their QA groups
                    # finish, so the drain chain is short
                    for g2 in range(0, NCH, 2):
                        nc.scalar.copy(
                            QAs[:, g2 : g2 + 2].rearrange("p g t q -> p (g t q)"),
                            QA[:, g2 : g2 + 2].rearrange("p g t q -> p (g t q)"),
                        )
                for g in range(NCH):
                    for t in range(NT):
                        nc.tensor.matmul(
                            QR[:, g, t, :], lhsT=Jx, rhs=QAs[:, g, t, :],
                            start=True, stop=True,
                        )

                # ---- per-batch assembly, split into half-batches so the
                # first half's chain (which only needs the first quad's
                # reductions) pre-executes during DVE's final instructions ----
                PI = spool.tile([128, NCH, 5, NT], f32, tag="pi")
                sh_tl = spool.tile([128, NCH, NT], f32, tag="shtl")
                sh_tr = spool.tile([128, NCH, NT], f32, tag="shtr")
                u = spool.tile([128, NCH, NT], f32, tag="u")
                v = spool.tile([128, NCH, NT], f32, tag="v")
                o_all = spool.tile([128, NT, 4 * NCH], f32, tag="oall")
                outv = out[b].rearrange("(t p) c -> p t c", p=128)

                nc.scalar.copy(
                    QB.rearrange("p g t (bb aa) -> p g t aa bb", bb=2, aa=2),
                    QDR[:],
                )

                for g0 in (0, NCH // 2):
                    gn = NCH // 2
                    gs = slice(g0, g0 + gn)

                    def bg_ap(base, tstep):
                        return bass.AP(
                            tensor=B_G.tensor,
                            offset=B_G[:, 0, 0, 0:1].offset + 16 * g0 + base,
                            ap=[B_G[:, 0, 0, 0:1].ap[0]]
                            + [[16, gn], [tstep, NT]],
                        )

                    def pi_ap(base, tstep, nt=NT):
                        return bass.AP(
                            tensor=PI.tensor,
                            offset=PI[:, 0, 0, 0:1].offset + 20 * g0 + base,
                            ap=[PI[:, 0, 0, 0:1].ap[0]]
                            + [[20, gn], [tstep, nt]],
                        )

                    nc.gpsimd.memset(PI[:, gs, 0, :], 0.0)
                    nc.gpsimd.tensor_copy(PI[:, gs, 1, :], B_G[:, gs, :, 0])
                    for m in range(2, 5):
                        nc.gpsimd.tensor_tensor(
                            PI[:, gs, m, :], PI[:, gs, m - 1, :],
                            B_G[:, gs, :, m - 1],
                            op=ADD,
                        )
                    # shell_tl = B[t][t] - RS1 + PI[m=t] + QB[col0]
                    nc.gpsimd.tensor_tensor(
                        sh_tl[:, gs], bg_ap(0, 5), RS[:, gs, :, 0], op=SUB
                    )
                    nc.gpsimd.tensor_tensor(
                        sh_tl[:, gs], sh_tl[:, gs], pi_ap(0, 5), op=ADD
                    )
                    nc.gpsimd.tensor_tensor(
                        sh_tl[:, gs], sh_tl[:, gs], QB[:, gs, :, 0], op=ADD
                    )
                    # shell_tr = B[t][3-t] - RS2 + S - PI[m=4-t] + QB[col1]
                    nc.gpsimd.tensor_tensor(
                        sh_tr[:, gs], bg_ap(3, 3), RS[:, gs, :, 1], op=SUB
                    )
                    nc.gpsimd.tensor_tensor(
                        sh_tr[:, gs], sh_tr[:, gs], pi_ap(16, 1), op=ADD
                    )
                    nc.gpsimd.tensor_tensor(
                        sh_tr[:, gs], sh_tr[:, gs], pi_ap(16, -3), op=SUB
                    )
                    nc.gpsimd.tensor_tensor(
                        sh_tr[:, gs], sh_tr[:, gs], QB[:, gs, :, 1], op=ADD
                    )
                    # br (src order): u = ST - sh_tl + S ; bl: v = STrev - sh_tr + S
                    nc.gpsimd.tensor_tensor(
                        u[:, gs], QB[:, gs, :, 2], sh_tl[:, gs], op=SUB
                    )
                    nc.gpsimd.tensor_tensor(u[:, gs], u[:, gs], pi_ap(16, 1), op=ADD)
                    nc.gpsimd.tensor_tensor(
                        v[:, gs], QB[:, gs, :, 3], sh_tr[:, gs], op=SUB
                    )
                    nc.gpsimd.tensor_tensor(v[:, gs], v[:, gs], pi_ap(16, 1), op=ADD)
                    for ci, (sr, wt) in enumerate(
                        [(sh_tl, wg), (sh_tr, wg), (v, wrevg), (u, wrevg)]
                    ):
                        nc.gpsimd.tensor_tensor(
                            o_all[:, :, ci * NCH + g0 : ci * NCH + g0 + gn],
                            sr[:, gs].rearrange("p g t -> p t g"),
                            wt[:, gs].rearrange("p g t -> p t g"),
                            op=MULT,
                        )

                pending_out.append((outv, o_all, nc.sync))

            while pending_out:
                dst, osrc, oeng = pending_out.pop(0)
                oeng.dma_start(out=dst, in_=osrc)
    nc.compile()
    return nc


def make_consts():
    r = np.arange(128)
    msu = (r[None, :] > r[:, None]).astype(np.float32)  # [c > r]
    manti = (r[None, :] + r[:, None] < 127).astype(np.float32)  # strict anti-tri
    J = (r[None, :] == 127 - r[:, None]).astype(np.float32)  # exchange matrix
    wq = np.zeros((128, 2, NT, NT, 2), np.float32)
    for t in range(NT):
        for tp in range(NT):
            if tp < t:
                wq[:, 0, t, tp, 0] = 1.0  # col-prefix above diag block
                wq[:, 1, t, tp, 0] = 1.0  # same, antidiag coords (pre-flip)
            wq[:, 0, t, tp, 1] = 1.0  # col total at col k
            wq[:, 1, t, tp, 1] = 1.0  # col total at col 511-k (pre-flip)
    e10 = np.zeros((128, 2), np.float32)
    e10[:, 0] = 1.0  # masked-product colsum -> col 0
    i_pt = (r[:, None] + 128 * np.arange(NT)[None, :]).astype(np.float64)
    w_pt = (1.0 / (2 * i_pt + 1)).astype(np.float32)  # [128, NT]
    wrev_pt = (1.0 / (1023.0 - 2 * i_pt)).astype(np.float32)
    wg = np.tile(w_pt[:, None, :], (1, NCH, 1)).astype(np.float32)
    wrevg = np.tile(wrev_pt[:, None, :], (1, NCH, 1)).astype(np.float32)
    cst = np.concatenate(
        [
            msu.reshape(128, -1),
            manti.reshape(128, -1),
            J.reshape(128, -1),
            wq.reshape(128, -1),
            e10.reshape(128, -1),
            wg.reshape(128, -1),
            wrevg.reshape(128, -1),
        ],
        axis=1,
    ).astype(np.float32)
    return dict(cst=cst)


_NC = None


def _get_nc():
    global _NC
    if _NC is None:
        _NC = build_nc()
    return _NC


def kernel(x: np.ndarray) -> np.ndarray:
    from concourse.bass_utils import run_bass_kernel_spmd

    x = np.asarray(x, dtype=np.float32)
    B = x.shape[0]
    consts = make_consts()
    per_core = B // N_CORES
    assert per_core == NB_CORE
    in_maps = [
        {"x": x[c * per_core : (c + 1) * per_core], **consts}
        for c in range(N_CORES)
    ]
    nc = _get_nc()
    res = run_bass_kernel_spmd(nc, in_maps, core_ids=list(range(N_CORES)))
    outs = []
    for r in res.results:
        o = r["out"].copy()  # [NB_CORE, 512, 4*NCH]
        o[:, :, 2 * NCH :] = o[:, ::-1, 2 * NCH :]
        outs.append(o)
    return np.concatenate(outs, axis=0)
